# revision 1
# baseline (speedup 1.0000x reference)
import numpy as np
import concourse.bass as bass
import concourse.mybir as mybir
from concourse.bass_utils import run_bass_kernel_spmd
from concourse.tile import TileContext
from concourse.mybir import AluOpType as Alu, ActivationFunctionType as Act

B, T, D, H, hd, SC, ST = 2, 2048, 1024, 16, 64, 64, 16
BT = B * T          # 4096
NC = 8
TOK = BT // NC      # 512 tokens per core
EPS = 1.1920929e-07
F32 = mybir.dt.float32


def _split_multi_waits(nc, max_waits=1):
    # this walrus build accepts only one sync wait per ISA instruction
    n = 0
    for f in nc.m.functions:
        for bb in f.blocks:
            out = []
            for inst in bb.instructions:
                si = inst.sync_info
                if si is not None and si.on_wait and len(si.on_wait) > max_waits:
                    for w in si.on_wait[:-max_waits]:
                        out.append(mybir.InstNoOp(
                            name=f"{inst.name}_ws{n}", ins=[], outs=[],
                            engine=inst.engine,
                            sync_info=mybir.SyncInfo(on_wait=[w], on_update=[]),
                            bass_nofuse=True))
                        n += 1
                    inst.sync_info = mybir.SyncInfo(
                        on_wait=si.on_wait[-max_waits:], on_update=si.on_update)
                out.append(inst)
            bb.instructions = out
    return n


def _build():
    nc = bass.Bass()
    dt = mybir.dt.float32r if int(__import__("os").environ.get("BASS_F32R", "1")) else F32

    xT = nc.dram_tensor("xT", [D, BT], dt, kind="ExternalInput")
    x_my = nc.dram_tensor("x_my", [TOK, D], dt, kind="ExternalInput")
    qkvwT = nc.dram_tensor("qkvwT", [D, 384], dt, kind="ExternalInput")
    o_wT = nc.dram_tensor("o_wT", [D, D], dt, kind="ExternalInput")
    out_wT = nc.dram_tensor("out_wT", [SC, D], dt, kind="ExternalInput")
    in_wT = nc.dram_tensor("in_wT", [D, SC], dt, kind="ExternalInput")
    gate_wT = nc.dram_tensor("gate_wT", [D, SC], dt, kind="ExternalInput")
    dt_wT = nc.dram_tensor("dt_wT", [SC, SC], dt, kind="ExternalInput")
    BpT = nc.dram_tensor("BpT", [SC, ST], dt, kind="ExternalInput")
    CpT = nc.dram_tensor("CpT", [SC, ST], dt, kind="ExternalInput")
    w1c = nc.dram_tensor("w1c", [128, 8], F32, kind="ExternalInput")
    w2t = nc.dram_tensor("w2t", [128, D], dt, kind="ExternalInput")
    dtb = nc.dram_tensor("dtb", [SC, 1], F32, kind="ExternalInput")
    alog = nc.dram_tensor("alog", [128, 8], F32, kind="ExternalInput")
    ident = nc.dram_tensor("ident", [128, 128], dt, kind="ExternalInput")
    tri = nc.dram_tensor("tri", [128, 128], dt, kind="ExternalInput")
    onesd = nc.dram_tensor("onesd", [128, 512], dt, kind="ExternalInput")
    esc = nc.dram_tensor("esc", [SC, 1024], dt, kind="ExternalInput")
    est = nc.dram_tensor("est", [ST, 128], dt, kind="ExternalInput")
    r8 = nc.dram_tensor("r8", [128, 512], dt, kind="ExternalInput")
    csel = nc.dram_tensor("csel", [128, 8], F32, kind="ExternalInput")
    omc = nc.dram_tensor("omc", [128, 64], F32, kind="ExternalInput")
    epsb = nc.dram_tensor("epsb", [128, 1], F32, kind="ExternalInput")

    yout = nc.dram_tensor("yout", [TOK, D], dt, kind="ExternalOutput")

    with nc.allow_low_precision(reason="float32r is fp32 bytes"), \
         TileContext(nc) as tc:
        with tc.tile_pool(name="const", bufs=1) as cpool, \
             tc.tile_pool(name="wts", bufs=1) as wpool, \
             tc.tile_pool(name="big", bufs=1) as bigp, \
             tc.tile_pool(name="work", bufs=2) as work, \
             tc.tile_pool(name="scan", bufs=1) as spool, \
             tc.tile_pool(name="psA", bufs=2, space="PSUM") as psA, \
             tc.tile_pool(name="psB", bufs=2, space="PSUM") as psB, \
             tc.tile_pool(name="psC", bufs=2, space="PSUM") as psC, \
             tc.tile_pool(name="dram", bufs=1, space="DRAM") as dram:

            def csbuf(shape, src, name, d=None):
                t = cpool.tile(shape, d or dt, name=name, tag=name)
                nc.sync.dma_start(t[:, :], src)
                return t

            identS = csbuf([128, 128], ident[:, :], "identS")
            triS = csbuf([128, 128], tri[:, :], "triS")
            onesS = csbuf([128, 512], onesd[:, :], "onesS")
            escS = csbuf([SC, 1024], esc[:, :], "escS")
            estS = csbuf([ST, 128], est[:, :], "estS")
            r8S = csbuf([128, 512], r8[:, :], "r8S")
            cselS = csbuf([128, 8], csel[:, :], "cselS", F32)
            omcS = csbuf([128, 64], omc[:, :], "omcS", F32)
            w1S = csbuf([128, 8], w1c[:, :], "w1S", F32)
            w2S = csbuf([128, D], w2t[:, :], "w2S")
            dtbS = csbuf([SC, 1], dtb[:, :], "dtbS", F32)
            alogS = csbuf([128, 8], alog[:, :], "alogS", F32)
            epsS = csbuf([128, 1], epsb[:, :], "epsS", F32)

            zerosF = cpool.tile([128, 512], F32, name="zerosF", tag="zerosF")
            nc.vector.memset(zerosF[:, :], 0.0)
            negA = cpool.tile([128, 8], F32, name="negA", tag="negA")
            nc.scalar.activation(negA[:, :], alogS[:, :], Act.Exp)
            nc.vector.tensor_scalar_mul(negA[:, :], negA[:, :], -1.0)

            qkvW = wpool.tile([128, 8 * 384], dt, name="qkvW", tag="qkvW")
            for k in range(8):
                nc.sync.dma_start(qkvW[:, k * 384:(k + 1) * 384],
                                  qkvwT[k * 128:(k + 1) * 128, :])
            inW = wpool.tile([128, 8 * SC], dt, name="inW", tag="inW")
            gateW = wpool.tile([128, 8 * SC], dt, name="gateW", tag="gateW")
            for k in range(8):
                nc.sync.dma_start(inW[:, k * SC:(k + 1) * SC],
                                  in_wT[k * 128:(k + 1) * 128, :])
                nc.sync.dma_start(gateW[:, k * SC:(k + 1) * SC],
                                  gate_wT[k * 128:(k + 1) * 128, :])
            outW = wpool.tile([SC, D], dt, name="outW", tag="outW")
            nc.sync.dma_start(outW[:, :], out_wT[:, :])
            dtW = wpool.tile([SC, SC], dt, name="dtW", tag="dtW")
            nc.sync.dma_start(dtW[:, :], dt_wT[:, :])
            BpS = wpool.tile([SC, ST], dt, name="BpS", tag="BpS")
            nc.sync.dma_start(BpS[:, :], BpT[:, :])
            CpS = wpool.tile([SC, ST], dt, name="CpS", tag="CpS")
            nc.sync.dma_start(CpS[:, :], CpT[:, :])

            cin = dram.tile([NC, 128, TOK], dt, name="cinT", tag="cinT")
            cout = dram.tile([NC, 128, TOK], dt, name="coutT", tag="coutT")

            # ======== per batch: rmsnorm1 + qkv + attention ========
            for b in range(B):
                Qf = bigp.tile([128, T], dt, tag="Qf", bufs=1)
                Kf = bigp.tile([128, T], dt, tag="Kf", bufs=1)
                Vp = [[bigp.tile([128, 65], dt, name=f"Vp{hh}_{kt}", tag=f"Vp{hh}_{kt}", bufs=1)
                       for kt in range(16)] for hh in range(2)]
                for hh in range(2):
                    for kt in range(16):
                        nc.vector.tensor_copy(Vp[hh][kt][:, 64:65], onesS[:, 0:1])

                for blk in range(4):  # 512-token blocks within batch
                    c0 = b * T + blk * 512
                    xt = [work.tile([128, 512], dt, name=f"xt{i}", tag="xt", bufs=8)
                          for i in range(8)]
                    ssp = psA.tile([1, 512], F32, tag="psA")
                    for k in range(8):
                        nc.sync.dma_start(xt[k][:, :],
                                          xT[k * 128:(k + 1) * 128, c0:c0 + 512])
                        sq = work.tile([128, 512], dt, tag="sq", bufs=2)
                        nc.scalar.activation(sq[:, :], xt[k][:, :], Act.Square)
                        nc.tensor.matmul(ssp[:, :], onesS[:, 0:1], sq[:, :],
                                         start=(k == 0), stop=(k == 7))
                    rs = work.tile([1, 512], dt, tag="rs")
                    nc.scalar.activation(rs[:, :], ssp[:, :], Act.Sqrt,
                                         scale=1.0 / D, bias=epsS[0:1, :])
                    nc.vector.reciprocal(rs[:, :], rs[:, :])
                    rsb = psA.tile([128, 512], F32, tag="psA")
                    nc.tensor.matmul(rsb[:, :], onesS[0:1, 0:128], rs[:, :],
                                     start=True, stop=True)
                    for k in range(8):
                        nc.vector.scalar_tensor_tensor(
                            xt[k][:, :], xt[k][:, :], w1S[:, k:k + 1],
                            rsb[:, :], Alu.mult, Alu.mult)
                    # qkv: m=0 Q, 1 K, 2 V
                    for m in range(3):
                        om = psB.tile([128, 512], F32, tag="psB")
                        for k in range(8):
                            nc.tensor.matmul(
                                om[:, :],
                                qkvW[:, k * 384 + m * 128:k * 384 + (m + 1) * 128],
                                xt[k][:, :], start=(k == 0), stop=(k == 7))
                        if m < 2:
                            dst = Qf if m == 0 else Kf
                            nc.scalar.copy(dst[:, blk * 512:(blk + 1) * 512], om[:, :])
                        else:
                            vfb = work.tile([128, 512], dt, tag="vfb")
                            nc.scalar.copy(vfb[:, :], om[:, :])
                            for sub in range(4):
                                kt = blk * 4 + sub
                                for hh in range(2):
                                    vtp = psC.tile([128, 64], dt, tag="psC")
                                    nc.tensor.transpose(
                                        vtp[:, :],
                                        vfb[64 * hh:64 * hh + 64,
                                            sub * 128:(sub + 1) * 128],
                                        identS[64 * hh:64 * hh + 64,
                                               64 * hh:64 * hh + 64])
                                    nc.vector.tensor_copy(Vp[hh][kt][:, 0:64],
                                                          vtp[:, :])

                # attention for this batch's two heads
                for hh in range(2):
                    r0 = 64 * hh
                    for qb in range(4):
                        q0 = qb * 512
                        ops = psC.tile([65, 512], F32, tag="psC")
                        nkt = 4 * qb + 4

                        def score_mm(kt):
                            sp = psB.tile([128, 512], F32, tag="psB")
                            nc.tensor.matmul(
                                sp[:, :],
                                Kf[r0:r0 + 64, kt * 128:(kt + 1) * 128],
                                Qf[r0:r0 + 64, q0:q0 + 512],
                                start=True, stop=True)
                            return sp

                        sps = score_mm(0)
                        for kt in range(nkt):
                            sp = sps
                            if kt + 1 < nkt:
                                sps = score_mm(kt + 1)
                            e = work.tile([128, 512], dt, tag="expst", bufs=3)
                            d = kt - 4 * qb
                            if d < 0:
                                nc.scalar.activation(e[:, :], sp[:, :], Act.Exp,
                                                     scale=0.125)
                            else:
                                if d > 0:
                                    nc.vector.tensor_copy(e[:, 0:128 * d],
                                                          zerosF[:, 0:128 * d])
                                nc.scalar.activation(e[:, 128 * d:512],
                                                     sp[:, 128 * d:512],
                                                     Act.Exp, scale=0.125)
                                nc.vector.tensor_mul(
                                    e[:, 128 * d:128 * (d + 1)],
                                    e[:, 128 * d:128 * (d + 1)], triS[:, :])
                            nc.tensor.matmul(ops[:, :], Vp[hh][kt][:, :], e[:, :],
                                             start=(kt == 0), stop=(kt == nkt - 1))
                        rl = work.tile([1, 512], dt, tag="rl")
                        nc.vector.reciprocal(rl[:, :], ops[64:65, :])
                        rb = psA.tile([64, 512], F32, tag="psA")
                        nc.tensor.matmul(rb[:, :], onesS[0:1, 0:64], rl[:, :],
                                         start=True, stop=True)
                        rbs = work.tile([64, 512], dt, tag="rbs")
                        nc.vector.tensor_copy(rbs[:, :], rb[:, :])
                        nc.vector.tensor_mul(rbs[:, :], ops[0:64, :], rbs[:, :])
                        nc.sync.dma_start(cin[4 * b + qb, r0:r0 + 64, :], rbs[:, :])

            nc.gpsimd.collective_compute(
                "AllToAll", Alu.bypass, [list(range(NC))],
                ins=[cin.opt()], outs=[cout.opt()])

            # ======== o_proj on my 512 tokens ========
            x1 = [bigp.tile([128, D], dt, name=f"x1_{ts}", tag=f"x1_{ts}", bufs=1)
                  for ts in range(4)]
            for ts in range(4):
                nc.sync.dma_start(x1[ts][:, :], x_my[ts * 128:(ts + 1) * 128, :])
            for m in range(8):
                pr = psB.tile([128, 512], F32, tag="psB")
                for k in range(8):
                    owt = work.tile([128, 128], dt, tag="owt", bufs=4)
                    nc.sync.dma_start(owt[:, :],
                                      o_wT[k * 128:(k + 1) * 128,
                                           m * 128:(m + 1) * 128])
                    otgk = work.tile([128, 512], dt, tag="otg", bufs=4)
                    nc.sync.dma_start(otgk[:, :], cout[k, :, :])
                    nc.tensor.matmul(pr[:, :], owt[:, :], otgk[:, :],
                                     start=(k == 0), stop=(k == 7))
                r1t = work.tile([128, 512], dt, tag="r1t")
                nc.scalar.copy(r1t[:, :], pr[:, :])
                for ts in range(4):
                    pxs = psA.tile([128, 128], dt, tag="psA")
                    nc.tensor.transpose(pxs[:, :],
                                        r1t[:, ts * 128:(ts + 1) * 128],
                                        identS[:, :])
                    nc.vector.tensor_add(x1[ts][:, m * 128:(m + 1) * 128],
                                         x1[ts][:, m * 128:(m + 1) * 128],
                                         pxs[:, :])

            # ======== rmsnorm2 (token-major) + transpose back ========
            h2T = [bigp.tile([128, 512], dt, name=f"h2T{k}", tag=f"h2T{k}", bufs=1)
                   for k in range(8)]
            for ts in range(4):
                h2 = work.tile([128, D], dt, tag="h2", bufs=1)
                ss2 = work.tile([128, 1], F32, tag="ss2")
                with nc.allow_low_precision(reason="float32r is fp32 bytes"):
                    nc.vector.scalar_tensor_tensor(h2[:, :], x1[ts][:, :], 1.0,
                                                   x1[ts][:, :], Alu.mult, Alu.mult,
                                                   accum_out=ss2[:, :])
                rq = work.tile([128, 1], F32, tag="rq")
                nc.scalar.activation(rq[:, :], ss2[:, :], Act.Sqrt,
                                     scale=1.0 / D, bias=epsS[:, :])
                nc.vector.reciprocal(rq[:, :], rq[:, :])
                nc.vector.scalar_tensor_tensor(h2[:, :], x1[ts][:, :], rq[:, :],
                                               w2S[:, :], Alu.mult, Alu.mult)
                for k in range(8):
                    ph = psC.tile([128, 128], dt, tag="psC")
                    nc.tensor.transpose(ph[:, :], h2[:, k * 128:(k + 1) * 128],
                                        identS[:, :])
                    nc.vector.tensor_copy(h2T[k][:, ts * 128:(ts + 1) * 128],
                                          ph[:, :])

            # ======== scan projections ========
            pz = psB.tile([SC, 512], F32, tag="psB")
            for k in range(8):
                nc.tensor.matmul(pz[:, :], inW[:, k * SC:(k + 1) * SC],
                                 h2T[k][:, :], start=(k == 0), stop=(k == 7))
            z_s = spool.tile([SC, 512], dt, tag="z_s")
            nc.vector.tensor_copy(z_s[:, :], pz[:, :])
            pg = psB.tile([SC, 512], F32, tag="psB")
            for k in range(8):
                nc.tensor.matmul(pg[:, :], gateW[:, k * SC:(k + 1) * SC],
                                 h2T[k][:, :], start=(k == 0), stop=(k == 7))
            gate_s = spool.tile([SC, 512], dt, tag="gate_s")
            nc.scalar.activation(gate_s[:, :], pg[:, :], Act.Silu)
            pdt = psB.tile([SC, 512], F32, tag="psB")
            nc.tensor.matmul(pdt[:, :], dtW[:, :], z_s[:, :], start=True, stop=True)
            dt_s = spool.tile([SC, 512], dt, tag="dt_s")
            nc.scalar.activation(dt_s[:, :], pdt[:, :], Act.Exp,
                                 bias=dtbS[:, :])
            nc.scalar.activation(dt_s[:, :], dt_s[:, :], Act.Ln, bias=1.0)
            dtz_s = spool.tile([SC, 512], dt, tag="dtz_s")
            nc.vector.tensor_mul(dtz_s[:, :], dt_s[:, :], z_s[:, :])
            pbi = psB.tile([ST, 512], F32, tag="psB")
            nc.tensor.matmul(pbi[:, :], BpS[:, :], z_s[:, :], start=True, stop=True)
            bi_s = spool.tile([ST, 512], dt, tag="bi_s")
            nc.vector.tensor_copy(bi_s[:, :], pbi[:, :])
            pci = psB.tile([ST, 512], F32, tag="psB")
            nc.tensor.matmul(pci[:, :], CpS[:, :], z_s[:, :], start=True, stop=True)
            ci_s = spool.tile([ST, 512], dt, tag="ci_s")
            nc.vector.tensor_copy(ci_s[:, :], pci[:, :])

            # ======== scan ========
            cin2 = dram.tile([128, 16], F32, name="cin2T", tag="cin2T")
            cout2 = dram.tile([NC * 128, 16], F32, name="cout2T", tag="cout2T")

            def scan_inputs(g):
                pde = psB.tile([128, 512], F32, tag="psB")
                nc.tensor.matmul(pde[:, :], escS[:, g * 128:(g + 1) * 128],
                                 dt_s[:, :], start=True, stop=True)
                abar = work.tile([128, 512], dt, tag="abar")
                nc.vector.scalar_tensor_tensor(abar[:, :], pde[:, :],
                                               negA[:, g:g + 1], onesS[:, :],
                                               Alu.mult, Alu.add)
                pdz = psB.tile([128, 512], F32, tag="psB")
                nc.tensor.matmul(pdz[:, :], escS[:, g * 128:(g + 1) * 128],
                                 dtz_s[:, :], start=True, stop=True)
                pbe = psC.tile([128, 512], F32, tag="psC")
                nc.tensor.matmul(pbe[:, :], estS[:, :], bi_s[:, :],
                                 start=True, stop=True)
                bes = work.tile([128, 512], dt, tag="bes")
                nc.vector.tensor_copy(bes[:, :], pbe[:, :])
                bin_ = work.tile([128, 512], dt, tag="bin_")
                nc.vector.tensor_mul(bin_[:, :], pdz[:, :], bes[:, :])
                return abar, bin_

            for g in range(8):
                abar, bin_ = scan_inputs(g)
                sc0 = work.tile([128, 512], dt, tag="sc0")
                nc.vector.tensor_tensor_scan(sc0[:, :], abar[:, :], bin_[:, :],
                                             0.0, Alu.mult, Alu.add)
                pp = work.tile([128, 512], dt, tag="pp")
                nc.vector.tensor_tensor_scan(pp[:, :], abar[:, :], abar[:, :],
                                             1.0, Alu.mult, Alu.bypass)
                nc.gpsimd.dma_start(cin2[:, g:g + 1], pp[:, 511:512])
                nc.gpsimd.dma_start(cin2[:, 8 + g:8 + g + 1], sc0[:, 511:512])

            nc.gpsimd.collective_compute(
                "AllGather", Alu.bypass, [list(range(NC))],
                ins=[cin2.opt()], outs=[cout2.opt()])

            sin = spool.tile([128, 8], F32, tag="sin")
            nc.vector.memset(sin[:, :], 0.0)
            for j in range(NC):
                pj = work.tile([128, 16], F32, tag="pj")
                nc.sync.dma_start(pj[:, :], cout2[j * 128:(j + 1) * 128, :])
                pe_ = work.tile([128, 8], F32, tag="pe_")
                nc.vector.scalar_tensor_tensor(pe_[:, :], pj[:, 0:8],
                                               cselS[:, j:j + 1],
                                               omcS[:, 8 * j:8 * j + 8],
                                               Alu.mult, Alu.add)
                se_ = work.tile([128, 8], F32, tag="se_")
                nc.vector.tensor_scalar_mul(se_[:, :], pj[:, 8:16],
                                            cselS[:, j:j + 1])
                nc.vector.tensor_mul(sin[:, :], sin[:, :], pe_[:, :])
                nc.vector.tensor_add(sin[:, :], sin[:, :], se_[:, :])

            yT = spool.tile([SC, 512], dt, tag="yT")
            pce = psC.tile([128, 512], F32, tag="psC")
            nc.tensor.matmul(pce[:, :], estS[:, :], ci_s[:, :], start=True, stop=True)
            ces = spool.tile([128, 512], dt, tag="ces")
            nc.vector.tensor_copy(ces[:, :], pce[:, :])
            py = psA.tile([SC, 512], F32, tag="psA")
            for g in range(8):
                abar, bin_ = scan_inputs(g)
                stc = work.tile([128, 512], dt, tag="stc")
                nc.vector.tensor_tensor_scan(stc[:, :], abar[:, :], bin_[:, :],
                                             sin[:, g:g + 1], Alu.mult, Alu.add)
                yt = work.tile([128, 512], dt, tag="yt")
                nc.vector.tensor_mul(yt[:, :], stc[:, :], ces[:, :])
                nc.tensor.matmul(py[:, :], r8S[:, g * 64:(g + 1) * 64], yt[:, :],
                                 start=(g == 0), stop=(g == 7))
            nc.vector.tensor_copy(yT[:, :], py[:, :])

            # ======== gate + out_proj + final residual ========
            yg = spool.tile([SC, 512], dt, tag="yg")
            nc.vector.tensor_mul(yg[:, :], yT[:, :], gate_s[:, :])
            for m in range(8):
                p2 = psB.tile([128, 512], F32, tag="psB")
                nc.tensor.matmul(p2[:, :], outW[:, m * 128:(m + 1) * 128],
                                 yg[:, :], start=True, stop=True)
                r2t = work.tile([128, 512], dt, tag="r2t")
                nc.scalar.copy(r2t[:, :], p2[:, :])
                for ts in range(4):
                    pxs = psA.tile([128, 128], dt, tag="psA")
                    nc.tensor.transpose(pxs[:, :],
                                        r2t[:, ts * 128:(ts + 1) * 128],
                                        identS[:, :])
                    nc.vector.tensor_add(x1[ts][:, m * 128:(m + 1) * 128],
                                         x1[ts][:, m * 128:(m + 1) * 128],
                                         pxs[:, :])
            for ts in range(4):
                nc.sync.dma_start(yout[ts * 128:(ts + 1) * 128, :], x1[ts][:, :])

    _split_multi_waits(nc)
    return nc


def kernel(x, qkv_w, o_w, norm1_w, norm2_w, in_w, out_w, A_log, Bp_w, Cp_w,
           dt_w, dt_b, gate_w):
    f = np.float32
    xf = np.ascontiguousarray(np.asarray(x, f).reshape(BT, D))
    xT = np.ascontiguousarray(xf.T)
    o_wT = np.ascontiguousarray(np.asarray(o_w, f).T)
    out_wT = np.ascontiguousarray(np.asarray(out_w, f).T)
    in_wT = np.ascontiguousarray(np.asarray(in_w, f).T)
    gate_wT = np.ascontiguousarray(np.asarray(gate_w, f).T)
    dt_wT = np.ascontiguousarray(np.asarray(dt_w, f).T)
    BpT = np.ascontiguousarray(np.asarray(Bp_w, f).T)
    CpT = np.ascontiguousarray(np.asarray(Cp_w, f).T)
    w1c = np.ascontiguousarray(np.asarray(norm1_w, f).reshape(8, 128).T)
    w2t = np.ascontiguousarray(np.tile(np.asarray(norm2_w, f)[None, :], (128, 1)))
    dtbv = np.ascontiguousarray(np.asarray(dt_b, f).reshape(SC, 1))
    alogv = np.ascontiguousarray(np.asarray(A_log, f).reshape(1024).reshape(8, 128).T)
    ident = np.eye(128, dtype=f)
    tri_m = (np.arange(128)[None, :] >= np.arange(128)[:, None]).astype(f)
    onesd = np.ones((128, 512), f)
    jj = np.arange(1024)
    escm = (np.arange(SC)[:, None] == (jj[None, :] // 16)).astype(f)
    estm = (np.arange(ST)[:, None] == (np.arange(128)[None, :] % 16)).astype(f)
    r8m = np.zeros((128, 512), f)
    for g in range(8):
        for j in range(128):
            r8m[j, g * 64 + 8 * g + j // 16] = 1.0

    nc = _build()
    in_maps = []
    for c in range(NC):
        b, q = c // 4, c % 4
        h0 = 2 * c
        rows = np.concatenate([np.arange(h0 * 64, (h0 + 2) * 64),
                               D + np.arange(h0 * 64, (h0 + 2) * 64),
                               2 * D + np.arange(h0 * 64, (h0 + 2) * 64)])
        qkvwT = np.ascontiguousarray(np.asarray(qkv_w, f)[rows, :].T)
        x_my = np.ascontiguousarray(xf[c * TOK:(c + 1) * TOK, :])
        sel = np.zeros(NC, f)
        for j in range(q):
            sel[4 * b + j] = 1.0
        cselv = np.ascontiguousarray(np.tile(sel[None, :], (128, 1)))
        omcv = np.ascontiguousarray(
            np.repeat(1.0 - sel, 8)[None, :].repeat(128, axis=0).astype(f))
        in_maps.append({
            "xT": xT, "x_my": x_my, "qkvwT": qkvwT, "o_wT": o_wT,
            "out_wT": out_wT, "in_wT": in_wT, "gate_wT": gate_wT,
            "dt_wT": dt_wT, "BpT": BpT, "CpT": CpT, "w1c": w1c, "w2t": w2t,
            "dtb": dtbv, "alog": alogv, "ident": ident, "tri": tri_m,
            "onesd": onesd, "esc": escm, "est": estm, "r8": r8m,
            "csel": cselv, "omc": omcv,
            "epsb": np.full((128, 1), EPS, f),
        })
    import os
    trace = bool(int(os.environ.get("BASS_PROFILE", "0")))
    res = run_bass_kernel_spmd(nc, in_maps, core_ids=list(range(NC)),
                               trace=trace)
    if trace:
        print("HW exec time:", res.exec_time_ns, "ns")
        print("trace:", res.instructions_and_trace[1] if res.instructions_and_trace else None)
    out = np.concatenate([res.results[c]["yout"] for c in range(NC)], axis=0)
    return out.reshape(B, T, D)



# revision 7
# speedup vs baseline: 1.5154x; 1.5154x over previous
import numpy as np
import concourse.bass as bass
import concourse.mybir as mybir
from concourse.bass_utils import run_bass_kernel_spmd
from concourse.tile import TileContext
from concourse.mybir import AluOpType as Alu, ActivationFunctionType as Act

B, T, D, H, hd, SC, ST = 2, 2048, 1024, 16, 64, 64, 16
BT = B * T          # 4096
NC = 8
TOK = BT // NC      # 512 tokens per core
EPS = 1.1920929e-07
F32 = mybir.dt.float32
BF16 = mybir.dt.bfloat16


def _split_multi_waits(nc, max_waits=1):
    # this walrus build accepts only one sync wait per ISA instruction
    n = 0
    for f in nc.m.functions:
        for bb in f.blocks:
            out = []
            for inst in bb.instructions:
                si = inst.sync_info
                if si is not None and si.on_wait and len(si.on_wait) > max_waits:
                    for w in si.on_wait[:-max_waits]:
                        out.append(mybir.InstNoOp(
                            name=f"{inst.name}_ws{n}", ins=[], outs=[],
                            engine=inst.engine,
                            sync_info=mybir.SyncInfo(on_wait=[w], on_update=[]),
                            bass_nofuse=True))
                        n += 1
                    inst.sync_info = mybir.SyncInfo(
                        on_wait=si.on_wait[-max_waits:], on_update=si.on_update)
                out.append(inst)
            bb.instructions = out
    return n


def _build():
    nc = bass.Bass()
    dt = mybir.dt.float32r if int(__import__("os").environ.get("BASS_F32R", "1")) else F32

    xT = nc.dram_tensor("xT", [D, BT], dt, kind="ExternalInput")
    xmyT = nc.dram_tensor("xmyT", [D, TOK], dt, kind="ExternalInput")
    qkvwT = nc.dram_tensor("qkvwT", [D, 384], dt, kind="ExternalInput")
    o_wTb = nc.dram_tensor("o_wTb", [D, D], BF16, kind="ExternalInput")
    out_wT = nc.dram_tensor("out_wT", [SC, D], dt, kind="ExternalInput")
    in_wT = nc.dram_tensor("in_wT", [D, SC], dt, kind="ExternalInput")
    gate_wT = nc.dram_tensor("gate_wT", [D, SC], dt, kind="ExternalInput")
    dt_wT = nc.dram_tensor("dt_wT", [SC, SC], dt, kind="ExternalInput")
    BpT = nc.dram_tensor("BpT", [SC, ST], dt, kind="ExternalInput")
    CpT = nc.dram_tensor("CpT", [SC, ST], dt, kind="ExternalInput")
    dtb = nc.dram_tensor("dtb", [SC, 1], F32, kind="ExternalInput")
    alog = nc.dram_tensor("alog", [128, 8], F32, kind="ExternalInput")
    ident = nc.dram_tensor("ident", [128, 128], dt, kind="ExternalInput")
    tri = nc.dram_tensor("tri", [128, 128], dt, kind="ExternalInput")
    onesd = nc.dram_tensor("onesd", [128, 128], dt, kind="ExternalInput")
    ej = nc.dram_tensor("ej", [8, 1024], dt, kind="ExternalInput")
    esc = nc.dram_tensor("esc", [SC, 1024], dt, kind="ExternalInput")
    est = nc.dram_tensor("est", [ST, 128], dt, kind="ExternalInput")
    r8 = nc.dram_tensor("r8", [128, 512], dt, kind="ExternalInput")
    csel = nc.dram_tensor("csel", [128, 8], F32, kind="ExternalInput")
    omc = nc.dram_tensor("omc", [128, 64], F32, kind="ExternalInput")
    epsb = nc.dram_tensor("epsb", [128, 1], F32, kind="ExternalInput")

    yout = nc.dram_tensor("yout", [D, TOK], dt, kind="ExternalOutput")

    with nc.allow_low_precision(reason="float32r is fp32 bytes; bf16 attn-out"), \
         TileContext(nc) as tc:
        with tc.tile_pool(name="const", bufs=1) as cpool, \
             tc.tile_pool(name="wts", bufs=1) as wpool, \
             tc.tile_pool(name="ow", bufs=1) as owp, \
             tc.tile_pool(name="xmy", bufs=1) as xmp, \
             tc.tile_pool(name="work", bufs=2) as work, \
             tc.tile_pool(name="psA", bufs=2, space="PSUM") as psA, \
             tc.tile_pool(name="psB", bufs=2, space="PSUM") as psB, \
             tc.tile_pool(name="psC", bufs=2, space="PSUM") as psC, \
             tc.tile_pool(name="dram", bufs=1, space="DRAM") as dram:

            def csbuf(shape, src, name, d=None):
                t = cpool.tile(shape, d or dt, name=name, tag=name)
                nc.sync.dma_start(t[:, :], src)
                return t

            identS = csbuf([128, 128], ident[:, :], "identS")
            triS = csbuf([128, 128], tri[:, :], "triS")
            onesS = csbuf([128, 128], onesd[:, :], "onesS")
            ejS = csbuf([8, 1024], ej[:, :], "ejS")
            escS = csbuf([SC, 1024], esc[:, :], "escS")
            estS = csbuf([ST, 128], est[:, :], "estS")
            r8S = csbuf([128, 512], r8[:, :], "r8S")
            cselS = csbuf([128, 8], csel[:, :], "cselS", F32)
            omcS = csbuf([128, 64], omc[:, :], "omcS", F32)
            dtbS = csbuf([SC, 1], dtb[:, :], "dtbS", F32)
            alogS = csbuf([128, 8], alog[:, :], "alogS", F32)
            epsS = csbuf([128, 1], epsb[:, :], "epsS", F32)

            zerosF = cpool.tile([128, 512], F32, name="zerosF", tag="zerosF")
            nc.vector.memset(zerosF[:, :], 0.0)
            negA = cpool.tile([128, 8], F32, name="negA", tag="negA")
            nc.scalar.activation(negA[:, :], alogS[:, :], Act.Exp)
            nc.vector.tensor_scalar_mul(negA[:, :], negA[:, :], -1.0)

            # ---- persistent weights ----
            inW = wpool.tile([128, 8 * SC], dt, name="inW", tag="inW")
            gateW = wpool.tile([128, 8 * SC], dt, name="gateW", tag="gateW")
            for k in range(8):
                nc.sync.dma_start(inW[:, k * SC:(k + 1) * SC],
                                  in_wT[k * 128:(k + 1) * 128, :])
                nc.sync.dma_start(gateW[:, k * SC:(k + 1) * SC],
                                  gate_wT[k * 128:(k + 1) * 128, :])
            outW = wpool.tile([SC, D], dt, name="outW", tag="outW")
            nc.sync.dma_start(outW[:, :], out_wT[:, :])
            dtW = wpool.tile([SC, SC], dt, name="dtW", tag="dtW")
            nc.sync.dma_start(dtW[:, :], dt_wT[:, :])
            BpS = wpool.tile([SC, ST], dt, name="BpS", tag="BpS")
            nc.sync.dma_start(BpS[:, :], BpT[:, :])
            CpS = wpool.tile([SC, ST], dt, name="CpS", tag="CpS")
            nc.sync.dma_start(CpS[:, :], CpT[:, :])
            # o_proj weights (bf16): needed only after the A2A; issue on the
            # gpsimd queue so they don't delay the x / qkv streams on sync
            oW = owp.tile([128, 8 * D], BF16, name="oW", tag="oW")
            for k in range(8):
                nc.gpsimd.dma_start(oW[:, k * D:(k + 1) * D],
                                    o_wTb[k * 128:(k + 1) * 128, :])

            # ---- dram collective buffers ----
            cin1 = dram.tile([1, 512], dt, name="cin1T", tag="cin1T")
            cout1 = dram.tile([NC, 512], dt, name="cout1T", tag="cout1T")
            cin = dram.tile([NC, 128, TOK], BF16, name="cinT", tag="cinT")
            cout = dram.tile([NC, 128, TOK], BF16, name="coutT", tag="coutT")
            cin2 = dram.tile([128, 16], F32, name="cin2T", tag="cin2T")
            cout2 = dram.tile([NC * 128, 16], F32, name="cout2T", tag="cout2T")

            # ---- my x tiles (D-major), kept for the two residual adds ----
            xtmy = [xmp.tile([128, 512], dt, name=f"xtmy{k}", tag=f"xtmy{k}")
                    for k in range(8)]
            for k in range(8):
                nc.sync.dma_start(xtmy[k][:, :], xmyT[k * 128:(k + 1) * 128, :])

            # ---- rmsnorm1 scale for my 512 tokens, allgather to all ----
            ssp = psA.tile([1, 512], F32, tag="psA")
            for k in range(8):
                sq = work.tile([128, 512], dt, tag="sq", bufs=2)
                nc.scalar.activation(sq[:, :], xtmy[k][:, :], Act.Square)
                nc.tensor.matmul(ssp[:, :], onesS[:, 0:1], sq[:, :],
                                 start=(k == 0), stop=(k == 7))
            # rsqrt(ms+eps) = exp(-0.5*ln(ms+eps)); stays in the exp/ln table
            lnm = work.tile([1, 512], F32, tag="lnm", bufs=1)
            nc.scalar.activation(lnm[:, :], ssp[:, :], Act.Ln,
                                 scale=1.0 / D, bias=epsS[0:1, :])
            srow = work.tile([1, 512], dt, tag="srow", bufs=1)
            nc.scalar.activation(srow[:, :], lnm[:, :], Act.Exp, scale=-0.5)
            nc.sync.dma_start(cin1[:, :], srow[:, :])
            nc.gpsimd.collective_compute(
                "AllGather", Alu.bypass, [list(range(NC))],
                ins=[cin1.opt()], outs=[cout1.opt()])
            sAG = wpool.tile([NC, 512], dt, name="sAG", tag="sAG")
            nc.sync.dma_start(sAG[:, :], cout1[:, :])

            # ---- qkv + attention (head-parallel over all tokens) ----
            with tc.tile_pool(name="attn", bufs=1) as apool:
                qkvW = apool.tile([128, 8 * 384], dt, name="qkvW", tag="qkvW")
                for k in range(8):
                    nc.sync.dma_start(qkvW[:, k * 384:(k + 1) * 384],
                                      qkvwT[k * 128:(k + 1) * 128, :])
                Qf = apool.tile([128, T], dt, name="Qf", tag="Qf")
                Kf = apool.tile([128, T], dt, name="Kf", tag="Kf")
                Vp = [[apool.tile([128, 65], dt, name=f"Vp{hh}_{kt}",
                                  tag=f"Vp{hh}_{kt}")
                       for kt in range(16)] for hh in range(2)]
                for hh in range(2):
                    for kt in range(16):
                        nc.vector.tensor_copy(Vp[hh][kt][:, 64:65], onesS[:, 0:1])

                for b in range(B):
                    for blk in range(4):
                        j = 4 * b + blk
                        rsp = psA.tile([128, 512], F32, tag="psA")
                        nc.tensor.matmul(rsp[:, :], ejS[:, j * 128:(j + 1) * 128],
                                         sAG[:, :], start=True, stop=True)
                        rsbS = apool.tile([128, 512], dt, tag="rsb", bufs=2)
                        nc.scalar.copy(rsbS[:, :], rsp[:, :])
                        xt = [apool.tile([128, 512], dt, name=f"xt{k}", tag="xt",
                                         bufs=16) for k in range(8)]
                        for k in range(8):
                            nc.sync.dma_start(xt[k][:, :],
                                              xT[k * 128:(k + 1) * 128,
                                                 j * 512:(j + 1) * 512])
                        for m in range(3):
                            om = psB.tile([128, 512], F32, tag="psB")
                            for k in range(8):
                                nc.tensor.matmul(
                                    om[:, :],
                                    qkvW[:, k * 384 + m * 128:k * 384 + (m + 1) * 128],
                                    xt[k][:, :], start=(k == 0), stop=(k == 7))
                            if m == 0:
                                nc.vector.tensor_mul(
                                    Qf[:, blk * 512:(blk + 1) * 512],
                                    om[:, :], rsbS[:, :])
                            elif m == 1:
                                nc.vector.tensor_mul(
                                    Kf[:, blk * 512:(blk + 1) * 512],
                                    om[:, :], rsbS[:, :])
                            else:
                                vfb = apool.tile([128, 512], dt, tag="vfb", bufs=2)
                                nc.vector.tensor_mul(vfb[:, :], om[:, :], rsbS[:, :])
                                for sub in range(4):
                                    kt = blk * 4 + sub
                                    for hh in range(2):
                                        vtp = psC.tile([128, 64], dt, tag="psC")
                                        nc.tensor.transpose(
                                            vtp[:, :],
                                            vfb[64 * hh:64 * hh + 64,
                                                sub * 128:(sub + 1) * 128],
                                            identS[64 * hh:64 * hh + 64,
                                                   64 * hh:64 * hh + 64])
                                        nc.vector.tensor_copy(
                                            Vp[hh][kt][:, 0:64], vtp[:, :])

                    # attention for this batch's two heads
                    for hh in range(2):
                        r0 = 64 * hh
                        for qb in range(4):
                            q0 = qb * 512
                            ops = psC.tile([65, 512], F32, tag="psC")
                            nkt = 4 * qb + 4

                            def score_mm(kt):
                                sp = psB.tile([128, 512], F32, tag="psB")
                                nc.tensor.matmul(
                                    sp[:, :],
                                    Kf[r0:r0 + 64, kt * 128:(kt + 1) * 128],
                                    Qf[r0:r0 + 64, q0:q0 + 512],
                                    start=True, stop=True)
                                return sp

                            sps = score_mm(0)
                            for kt in range(nkt):
                                sp = sps
                                if kt + 1 < nkt:
                                    sps = score_mm(kt + 1)
                                e = apool.tile([128, 512], dt, tag="expst", bufs=3)
                                d = kt - 4 * qb
                                if d < 0:
                                    nc.scalar.activation(e[:, :], sp[:, :], Act.Exp,
                                                         scale=0.125)
                                else:
                                    if d > 0:
                                        nc.vector.tensor_copy(e[:, 0:128 * d],
                                                              zerosF[:, 0:128 * d])
                                    nc.scalar.activation(e[:, 128 * d:512],
                                                         sp[:, 128 * d:512],
                                                         Act.Exp, scale=0.125)
                                    nc.vector.tensor_mul(
                                        e[:, 128 * d:128 * (d + 1)],
                                        e[:, 128 * d:128 * (d + 1)], triS[:, :])
                                nc.tensor.matmul(ops[:, :], Vp[hh][kt][:, :], e[:, :],
                                                 start=(kt == 0), stop=(kt == nkt - 1))
                            # 1/denominator = exp(-ln(den)) on Act engine
                            lnd = apool.tile([1, 512], F32, tag="lnd", bufs=2)
                            nc.scalar.activation(lnd[:, :], ops[64:65, :], Act.Ln)
                            rle = apool.tile([1, 512], dt, tag="rle", bufs=2)
                            nc.scalar.activation(rle[:, :], lnd[:, :], Act.Exp,
                                                 scale=-1.0)
                            rb = psA.tile([64, 512], F32, tag="psA")
                            nc.tensor.matmul(rb[:, :], onesS[0:1, 0:64], rle[:, :],
                                             start=True, stop=True)
                            rbc = apool.tile([64, 512], dt, tag="rbc", bufs=2)
                            nc.vector.tensor_copy(rbc[:, :], rb[:, :])
                            rbs = apool.tile([64, 512], BF16, tag="rbs", bufs=2)
                            nc.vector.tensor_mul(rbs[:, :], ops[0:64, :], rbc[:, :])
                            nc.sync.dma_start(cin[4 * b + qb, r0:r0 + 64, :],
                                              rbs[:, :])

            nc.gpsimd.collective_compute(
                "AllToAll", Alu.bypass, [list(range(NC))],
                ins=[cin.opt()], outs=[cout.opt()])

            with tc.tile_pool(name="xpool", bufs=1) as xpool:
                x1d = [xpool.tile([128, 512], dt, name=f"x1d{m}", tag=f"x1d{m}")
                       for m in range(8)]
                h2T = [xpool.tile([128, 512], dt, name=f"h2T{k}", tag=f"h2T{k}")
                       for k in range(8)]

                with tc.tile_pool(name="postA", bufs=1) as pA:
                    # attention rows for my tokens, loaded once (bf16)
                    otg = [pA.tile([128, 512], BF16, name=f"otg{k}", tag=f"otg{k}")
                           for k in range(8)]
                    for k in range(8):
                        nc.sync.dma_start(otg[k][:, :], cout[k, :, :])
                    # o_proj + residual (D-major)
                    for m in range(8):
                        pr = psB.tile([128, 512], F32, tag="psB")
                        for k in range(8):
                            nc.tensor.matmul(
                                pr[:, :],
                                oW[:, k * D + m * 128:k * D + (m + 1) * 128],
                                otg[k][:, :], start=(k == 0), stop=(k == 7))
                        nc.vector.tensor_add(x1d[m][:, :], pr[:, :], xtmy[m][:, :])

                # ---- rmsnorm2 (D-major) ----
                ssp2 = psA.tile([1, 512], F32, tag="psA")
                for k in range(8):
                    sq2 = work.tile([128, 512], dt, tag="sq", bufs=2)
                    nc.scalar.activation(sq2[:, :], x1d[k][:, :], Act.Square)
                    nc.tensor.matmul(ssp2[:, :], onesS[:, 0:1], sq2[:, :],
                                     start=(k == 0), stop=(k == 7))
                ln2 = work.tile([1, 512], F32, tag="lnm", bufs=1)
                nc.scalar.activation(ln2[:, :], ssp2[:, :], Act.Ln,
                                     scale=1.0 / D, bias=epsS[0:1, :])
                s2row = work.tile([1, 512], dt, tag="srow", bufs=1)
                nc.scalar.activation(s2row[:, :], ln2[:, :], Act.Exp, scale=-0.5)
                rs2p = psA.tile([128, 512], F32, tag="psA")
                nc.tensor.matmul(rs2p[:, :], onesS[0:1, 0:128], s2row[:, :],
                                 start=True, stop=True)
                rsb2S = work.tile([128, 512], dt, tag="rsb2", bufs=1)
                nc.scalar.copy(rsb2S[:, :], rs2p[:, :])
                for k in range(8):
                    nc.vector.tensor_mul(h2T[k][:, :], x1d[k][:, :], rsb2S[:, :])

                with tc.tile_pool(name="postB", bufs=1) as pB:
                    # ---- scan projections ----
                    pz = psB.tile([SC, 512], F32, tag="psB")
                    for k in range(8):
                        nc.tensor.matmul(pz[:, :], inW[:, k * SC:(k + 1) * SC],
                                         h2T[k][:, :], start=(k == 0), stop=(k == 7))
                    z_s = pB.tile([SC, 512], dt, name="z_s", tag="z_s")
                    nc.vector.tensor_copy(z_s[:, :], pz[:, :])
                    pdt = psB.tile([SC, 512], F32, tag="psB")
                    nc.tensor.matmul(pdt[:, :], dtW[:, :], z_s[:, :],
                                     start=True, stop=True)
                    dt_s = pB.tile([SC, 512], dt, name="dt_s", tag="dt_s")
                    nc.scalar.activation(dt_s[:, :], pdt[:, :], Act.Exp,
                                         bias=dtbS[:, :])
                    nc.scalar.activation(dt_s[:, :], dt_s[:, :], Act.Ln, bias=1.0)
                    dtz_s = pB.tile([SC, 512], dt, name="dtz_s", tag="dtz_s")
                    nc.vector.tensor_mul(dtz_s[:, :], dt_s[:, :], z_s[:, :])
                    pbi = psB.tile([ST, 512], F32, tag="psB")
                    nc.tensor.matmul(pbi[:, :], BpS[:, :], z_s[:, :],
                                     start=True, stop=True)
                    bi_s = pB.tile([ST, 512], dt, name="bi_s", tag="bi_s")
                    nc.vector.tensor_copy(bi_s[:, :], pbi[:, :])
                    pci = psB.tile([ST, 512], F32, tag="psB")
                    nc.tensor.matmul(pci[:, :], CpS[:, :], z_s[:, :],
                                     start=True, stop=True)
                    ci_s = pB.tile([ST, 512], dt, name="ci_s", tag="ci_s")
                    nc.vector.tensor_copy(ci_s[:, :], pci[:, :])

                    # ---- local scans (zero-init trajectories + cumprods) ----
                    ppT = [pB.tile([128, 512], BF16, name=f"ppT{g}", tag=f"ppT{g}")
                           for g in range(8)]
                    sc0T = [pB.tile([128, 512], dt, name=f"sc0T{g}", tag=f"sc0T{g}")
                            for g in range(8)]
                    for g in range(8):
                        pde = psB.tile([128, 512], F32, tag="psB")
                        nc.tensor.matmul(pde[:, :], escS[:, g * 128:(g + 1) * 128],
                                         dt_s[:, :], start=True, stop=True)
                        abar = pB.tile([128, 512], dt, tag="abar", bufs=2)
                        nc.scalar.activation(abar[:, :], pde[:, :], Act.Identity,
                                             scale=negA[:, g:g + 1], bias=1.0)
                        pdz = psB.tile([128, 512], F32, tag="psB")
                        nc.tensor.matmul(pdz[:, :], escS[:, g * 128:(g + 1) * 128],
                                         dtz_s[:, :], start=True, stop=True)
                        pbe = psC.tile([128, 512], F32, tag="psC")
                        nc.tensor.matmul(pbe[:, :], estS[:, :], bi_s[:, :],
                                         start=True, stop=True)
                        bes = pB.tile([128, 512], dt, tag="bes", bufs=2)
                        nc.scalar.copy(bes[:, :], pbe[:, :])
                        bin_ = pB.tile([128, 512], dt, tag="bin_", bufs=2)
                        nc.vector.tensor_mul(bin_[:, :], pdz[:, :], bes[:, :])
                        nc.vector.tensor_tensor_scan(sc0T[g][:, :], abar[:, :],
                                                     bin_[:, :], 0.0,
                                                     Alu.mult, Alu.add)
                        nc.vector.tensor_tensor_scan(ppT[g][:, :], abar[:, :],
                                                     abar[:, :], 1.0,
                                                     Alu.mult, Alu.bypass)
                        nc.gpsimd.dma_start(cin2[:, g:g + 1], ppT[g][:, 511:512])
                        nc.gpsimd.dma_start(cin2[:, 8 + g:8 + g + 1],
                                            sc0T[g][:, 511:512])

                    nc.gpsimd.collective_compute(
                        "AllGather", Alu.bypass, [list(range(NC))],
                        ins=[cin2.opt()], outs=[cout2.opt()])

                    # overlap the AllGather with work that doesn't need it:
                    # C embedding and the gate (single act-table switch to Silu)
                    ces = pB.tile([128, 512], dt, name="ces", tag="ces")
                    pce = psC.tile([128, 512], F32, tag="psC")
                    nc.tensor.matmul(pce[:, :], estS[:, :], ci_s[:, :],
                                     start=True, stop=True)
                    nc.vector.tensor_copy(ces[:, :], pce[:, :])
                    pg = psB.tile([SC, 512], F32, tag="psB")
                    for k in range(8):
                        nc.tensor.matmul(pg[:, :], gateW[:, k * SC:(k + 1) * SC],
                                         h2T[k][:, :], start=(k == 0), stop=(k == 7))
                    gate_s = pB.tile([SC, 512], dt, name="gate_s", tag="gate_s")
                    nc.scalar.activation(gate_s[:, :], pg[:, :], Act.Silu)

                    # ---- stitch initial states from preceding cores ----
                    sin = pB.tile([128, 8], F32, name="sin", tag="sin")
                    nc.vector.memset(sin[:, :], 0.0)
                    for jj in range(NC):
                        pj = work.tile([128, 16], F32, tag="pj")
                        nc.sync.dma_start(pj[:, :],
                                          cout2[jj * 128:(jj + 1) * 128, :])
                        pe_ = work.tile([128, 8], F32, tag="pe_")
                        nc.vector.scalar_tensor_tensor(pe_[:, :], pj[:, 0:8],
                                                       cselS[:, jj:jj + 1],
                                                       omcS[:, 8 * jj:8 * jj + 8],
                                                       Alu.mult, Alu.add)
                        se_ = work.tile([128, 8], F32, tag="se_")
                        nc.vector.tensor_scalar_mul(se_[:, :], pj[:, 8:16],
                                                    cselS[:, jj:jj + 1])
                        nc.vector.tensor_mul(sin[:, :], sin[:, :], pe_[:, :])
                        nc.vector.tensor_add(sin[:, :], sin[:, :], se_[:, :])

                    # ---- finalize: state = sc0 + sin * pp  (linearity) ----
                    py = psA.tile([SC, 512], F32, tag="psA")
                    for g in range(8):
                        stc = pB.tile([128, 512], dt, tag="stc", bufs=2)
                        nc.vector.scalar_tensor_tensor(stc[:, :], ppT[g][:, :],
                                                       sin[:, g:g + 1],
                                                       sc0T[g][:, :],
                                                       Alu.mult, Alu.add)
                        yt = pB.tile([128, 512], dt, tag="yt", bufs=2)
                        nc.vector.tensor_mul(yt[:, :], stc[:, :], ces[:, :])
                        nc.tensor.matmul(py[:, :], r8S[:, g * 64:(g + 1) * 64],
                                         yt[:, :], start=(g == 0), stop=(g == 7))
                    yT = pB.tile([SC, 512], dt, name="yT", tag="yT")
                    nc.vector.tensor_copy(yT[:, :], py[:, :])

                    # ---- gate + out_proj + final residual (D-major) ----
                    yg = pB.tile([SC, 512], dt, name="yg", tag="yg")
                    nc.vector.tensor_mul(yg[:, :], yT[:, :], gate_s[:, :])
                    for m in range(8):
                        p2 = psB.tile([128, 512], F32, tag="psB")
                        nc.tensor.matmul(p2[:, :], outW[:, m * 128:(m + 1) * 128],
                                         yg[:, :], start=True, stop=True)
                        yo = pB.tile([128, 512], dt, tag="yo", bufs=2)
                        nc.vector.tensor_add(yo[:, :], p2[:, :], x1d[m][:, :])
                        nc.sync.dma_start(yout[m * 128:(m + 1) * 128, :], yo[:, :])

    _split_multi_waits(nc)
    return nc


def kernel(x, qkv_w, o_w, norm1_w, norm2_w, in_w, out_w, A_log, Bp_w, Cp_w,
           dt_w, dt_b, gate_w):
    import ml_dtypes
    f = np.float32
    bf = ml_dtypes.bfloat16
    xf = np.ascontiguousarray(np.asarray(x, f).reshape(BT, D))
    xT = np.ascontiguousarray(xf.T)
    # fold the rmsnorm elementwise weights into the consuming projections
    qkv_w1 = np.asarray(qkv_w, f) * np.asarray(norm1_w, f)[None, :]
    in_w2 = np.asarray(in_w, f) * np.asarray(norm2_w, f)[None, :]
    gate_w2 = np.asarray(gate_w, f) * np.asarray(norm2_w, f)[None, :]
    o_wTb = np.ascontiguousarray(np.asarray(o_w, f).T.astype(bf))
    out_wT = np.ascontiguousarray(np.asarray(out_w, f).T)
    in_wT = np.ascontiguousarray(in_w2.T)
    gate_wT = np.ascontiguousarray(gate_w2.T)
    dt_wT = np.ascontiguousarray(np.asarray(dt_w, f).T)
    BpT = np.ascontiguousarray(np.asarray(Bp_w, f).T)
    CpT = np.ascontiguousarray(np.asarray(Cp_w, f).T)
    dtbv = np.ascontiguousarray(np.asarray(dt_b, f).reshape(SC, 1))
    alogv = np.ascontiguousarray(np.asarray(A_log, f).reshape(1024).reshape(8, 128).T)
    ident = np.eye(128, dtype=f)
    tri_m = (np.arange(128)[None, :] >= np.arange(128)[:, None]).astype(f)
    onesd = np.ones((128, 128), f)
    ejm = np.zeros((8, 1024), f)
    for j in range(8):
        ejm[j, j * 128:(j + 1) * 128] = 1.0
    jj = np.arange(1024)
    escm = (np.arange(SC)[:, None] == (jj[None, :] // 16)).astype(f)
    estm = (np.arange(ST)[:, None] == (np.arange(128)[None, :] % 16)).astype(f)
    r8m = np.zeros((128, 512), f)
    for g in range(8):
        for j in range(128):
            r8m[j, g * 64 + 8 * g + j // 16] = 1.0

    nc = _build()
    in_maps = []
    for c in range(NC):
        b, q = c // 4, c % 4
        h0 = 2 * c
        rows = np.concatenate([np.arange(h0 * 64, (h0 + 2) * 64),
                               D + np.arange(h0 * 64, (h0 + 2) * 64),
                               2 * D + np.arange(h0 * 64, (h0 + 2) * 64)])
        qkvwT = np.ascontiguousarray(qkv_w1[rows, :].T)
        xmyT = np.ascontiguousarray(xT[:, c * TOK:(c + 1) * TOK])
        sel = np.zeros(NC, f)
        for j in range(q):
            sel[4 * b + j] = 1.0
        cselv = np.ascontiguousarray(np.tile(sel[None, :], (128, 1)))
        omcv = np.ascontiguousarray(
            np.repeat(1.0 - sel, 8)[None, :].repeat(128, axis=0).astype(f))
        in_maps.append({
            "xT": xT, "xmyT": xmyT, "qkvwT": qkvwT, "o_wTb": o_wTb,
            "out_wT": out_wT, "in_wT": in_wT, "gate_wT": gate_wT,
            "dt_wT": dt_wT, "BpT": BpT, "CpT": CpT,
            "dtb": dtbv, "alog": alogv, "ident": ident, "tri": tri_m,
            "onesd": onesd, "ej": ejm, "esc": escm, "est": estm, "r8": r8m,
            "csel": cselv, "omc": omcv,
            "epsb": np.full((128, 1), EPS, f),
        })
    import os
    trace = bool(int(os.environ.get("BASS_PROFILE", "0")))
    res = run_bass_kernel_spmd(nc, in_maps, core_ids=list(range(NC)),
                               trace=trace)
    if trace:
        print("HW exec time:", res.exec_time_ns, "ns")
        print("trace:", res.instructions_and_trace[1] if res.instructions_and_trace else None)
    out = np.concatenate([res.results[c]["yout"].T for c in range(NC)], axis=0)
    return out.reshape(B, T, D)


# revision 14
# speedup vs baseline: 1.7176x; 1.1334x over previous
import numpy as np
import concourse.bass as bass
import concourse.mybir as mybir
from concourse.bass_utils import run_bass_kernel_spmd
from concourse.tile import TileContext
from concourse.mybir import AluOpType as Alu, ActivationFunctionType as Act

B, T, D, H, hd, SC, ST = 2, 2048, 1024, 16, 64, 64, 16
BT = B * T          # 4096
NC = 8
TOK = BT // NC      # 512 tokens per core
EPS = 1.1920929e-07
F32 = mybir.dt.float32
BF16 = mybir.dt.bfloat16


def _split_multi_waits(nc, max_waits=1):
    # this walrus build accepts only one sync wait per ISA instruction
    n = 0
    for f in nc.m.functions:
        for bb in f.blocks:
            out = []
            for inst in bb.instructions:
                si = inst.sync_info
                if si is not None and si.on_wait and len(si.on_wait) > max_waits:
                    for w in si.on_wait[:-max_waits]:
                        out.append(mybir.InstNoOp(
                            name=f"{inst.name}_ws{n}", ins=[], outs=[],
                            engine=inst.engine,
                            sync_info=mybir.SyncInfo(on_wait=[w], on_update=[]),
                            bass_nofuse=True))
                        n += 1
                    inst.sync_info = mybir.SyncInfo(
                        on_wait=si.on_wait[-max_waits:], on_update=si.on_update)
                out.append(inst)
            bb.instructions = out
    return n


def _build():
    nc = bass.Bass()
    dt = mybir.dt.float32r if int(__import__("os").environ.get("BASS_F32R", "1")) else F32

    xT = nc.dram_tensor("xT", [D, BT], BF16, kind="ExternalInput")
    xmyT = nc.dram_tensor("xmyT", [D, TOK], dt, kind="ExternalInput")
    qkvwT = nc.dram_tensor("qkvwT", [D, 384], BF16, kind="ExternalInput")
    o_wTb = nc.dram_tensor("o_wTb", [D, D], BF16, kind="ExternalInput")
    out_wT = nc.dram_tensor("out_wT", [SC, D], dt, kind="ExternalInput")
    in_wT = nc.dram_tensor("in_wT", [D, SC], dt, kind="ExternalInput")
    gate_wT = nc.dram_tensor("gate_wT", [D, SC], dt, kind="ExternalInput")
    dt_wT = nc.dram_tensor("dt_wT", [SC, SC], dt, kind="ExternalInput")
    BpT = nc.dram_tensor("BpT", [SC, ST], dt, kind="ExternalInput")
    CpT = nc.dram_tensor("CpT", [SC, ST], dt, kind="ExternalInput")
    dtb = nc.dram_tensor("dtb", [SC, 1], F32, kind="ExternalInput")
    alog = nc.dram_tensor("alog", [128, 8], F32, kind="ExternalInput")
    ident = nc.dram_tensor("ident", [128, 128], BF16, kind="ExternalInput")
    tri = nc.dram_tensor("tri", [128, 128], BF16, kind="ExternalInput")
    onesd = nc.dram_tensor("onesd", [128, 128], dt, kind="ExternalInput")
    ej = nc.dram_tensor("ej", [8, 1024], dt, kind="ExternalInput")
    esc = nc.dram_tensor("esc", [SC, 1024], dt, kind="ExternalInput")
    est = nc.dram_tensor("est", [ST, 128], dt, kind="ExternalInput")
    r8 = nc.dram_tensor("r8", [128, 512], dt, kind="ExternalInput")
    csel = nc.dram_tensor("csel", [128, 8], F32, kind="ExternalInput")
    omc = nc.dram_tensor("omc", [128, 64], F32, kind="ExternalInput")
    epsb = nc.dram_tensor("epsb", [128, 1], F32, kind="ExternalInput")

    yout = nc.dram_tensor("yout", [D, TOK], dt, kind="ExternalOutput")

    with nc.allow_low_precision(reason="float32r is fp32 bytes; bf16 attn-out"), \
         TileContext(nc) as tc:
        with tc.tile_pool(name="const", bufs=1) as cpool, \
             tc.tile_pool(name="wts", bufs=1) as wpool, \
             tc.tile_pool(name="ow", bufs=1) as owp, \
             tc.tile_pool(name="xmy", bufs=1) as xmp, \
             tc.tile_pool(name="work", bufs=2) as work, \
             tc.tile_pool(name="psA", bufs=2, space="PSUM") as psA, \
             tc.tile_pool(name="psB", bufs=2, space="PSUM") as psB, \
             tc.tile_pool(name="psC", bufs=2, space="PSUM") as psC, \
             tc.tile_pool(name="dram", bufs=1, space="DRAM") as dram:

            def csbuf(shape, src, name, d=None):
                t = cpool.tile(shape, d or dt, name=name, tag=name)
                nc.sync.dma_start(t[:, :], src)
                return t

            identS = csbuf([128, 128], ident[:, :], "identS", BF16)
            triS = csbuf([128, 128], tri[:, :], "triS", BF16)
            onesS = csbuf([128, 128], onesd[:, :], "onesS")
            ejS = csbuf([8, 1024], ej[:, :], "ejS")
            escS = csbuf([SC, 1024], esc[:, :], "escS")
            estS = csbuf([ST, 128], est[:, :], "estS")
            r8S = csbuf([128, 512], r8[:, :], "r8S")
            cselS = csbuf([128, 8], csel[:, :], "cselS", F32)
            omcS = csbuf([128, 64], omc[:, :], "omcS", F32)
            dtbS = csbuf([SC, 1], dtb[:, :], "dtbS", F32)
            alogS = csbuf([128, 8], alog[:, :], "alogS", F32)
            epsS = csbuf([128, 1], epsb[:, :], "epsS", F32)

            zerosF = cpool.tile([128, 512], F32, name="zerosF", tag="zerosF")
            nc.vector.memset(zerosF[:, :], 0.0)
            negA = cpool.tile([128, 8], F32, name="negA", tag="negA")
            nc.scalar.activation(negA[:, :], alogS[:, :], Act.Exp)
            nc.vector.tensor_scalar_mul(negA[:, :], negA[:, :], -1.0)

            # ---- persistent weight tiles (DMAs for the post-A2A weights are
            # issued after the attention section so they don't delay the
            # x / qkv streams on the sync queue) ----
            inW = wpool.tile([128, 8 * SC], dt, name="inW", tag="inW")
            gateW = wpool.tile([128, 8 * SC], dt, name="gateW", tag="gateW")
            outW = wpool.tile([SC, D], dt, name="outW", tag="outW")
            dtW = wpool.tile([SC, SC], dt, name="dtW", tag="dtW")
            BpS = wpool.tile([SC, ST], dt, name="BpS", tag="BpS")
            CpS = wpool.tile([SC, ST], dt, name="CpS", tag="CpS")
            # o_proj weights (bf16) on the gpsimd queue
            oW = owp.tile([128, 8 * D], BF16, name="oW", tag="oW")
            for k in range(8):
                nc.gpsimd.dma_start(oW[:, k * D:(k + 1) * D],
                                    o_wTb[k * 128:(k + 1) * 128, :])

            # ---- dram collective buffers ----
            cin1 = dram.tile([1, 512], dt, name="cin1T", tag="cin1T")
            cout1 = dram.tile([NC, 512], dt, name="cout1T", tag="cout1T")
            cin = dram.tile([NC, 128, TOK], BF16, name="cinT", tag="cinT")
            cout = dram.tile([NC, 128, TOK], BF16, name="coutT", tag="coutT")
            cin2 = dram.tile([128, 16], F32, name="cin2T", tag="cin2T")
            cout2 = dram.tile([NC * 128, 16], F32, name="cout2T", tag="cout2T")

            # ---- my x tiles (D-major), kept for the two residual adds ----
            xtmy = [xmp.tile([128, 512], dt, name=f"xtmy{k}", tag=f"xtmy{k}")
                    for k in range(8)]
            for k in range(8):
                nc.sync.dma_start(xtmy[k][:, :], xmyT[k * 128:(k + 1) * 128, :])

            # ---- rmsnorm1 scale for my 512 tokens, allgather to all ----
            ssp = psA.tile([1, 512], F32, tag="psA")
            for k in range(8):
                sq = work.tile([128, 512], dt, tag="sq", bufs=2)
                nc.scalar.activation(sq[:, :], xtmy[k][:, :], Act.Square)
                nc.tensor.matmul(ssp[:, :], onesS[:, 0:1], sq[:, :],
                                 start=(k == 0), stop=(k == 7))
            # rsqrt(ms+eps) = exp(-0.5*ln(ms+eps)); stays in the exp/ln table
            lnm = work.tile([1, 512], F32, tag="lnm", bufs=1)
            nc.scalar.activation(lnm[:, :], ssp[:, :], Act.Ln,
                                 scale=1.0 / D, bias=epsS[0:1, :])
            srow = work.tile([1, 512], dt, tag="srow", bufs=1)
            nc.scalar.activation(srow[:, :], lnm[:, :], Act.Exp, scale=-0.5)
            # keep the collective-dependent transfers off the sync queue so
            # the qkv weight / x streams behind them are not stalled
            nc.gpsimd.dma_start(cin1[:, :], srow[:, :])
            nc.gpsimd.collective_compute(
                "AllGather", Alu.bypass, [list(range(NC))],
                ins=[cin1.opt()], outs=[cout1.opt()])
            sAG = wpool.tile([NC, 512], dt, name="sAG", tag="sAG")
            nc.gpsimd.dma_start(sAG[:, :], cout1[:, :])

            # ---- qkv + attention (head-parallel over all tokens) ----
            with tc.tile_pool(name="attn", bufs=1) as apool:
                qkvW = apool.tile([128, 8 * 384], BF16, name="qkvW", tag="qkvW")
                for k in range(8):
                    nc.sync.dma_start(qkvW[:, k * 384:(k + 1) * 384],
                                      qkvwT[k * 128:(k + 1) * 128, :])
                Qf = apool.tile([128, T], BF16, name="Qf", tag="Qf")
                Kf = apool.tile([128, T], BF16, name="Kf", tag="Kf")
                Vp = [[apool.tile([128, 65], BF16, name=f"Vp{hh}_{kt}",
                                  tag=f"Vp{hh}_{kt}")
                       for kt in range(16)] for hh in range(2)]
                for hh in range(2):
                    for kt in range(16):
                        nc.vector.tensor_copy(Vp[hh][kt][:, 64:65], onesS[:, 0:1])

                for b in range(B):
                    for blk in range(4):
                        j = 4 * b + blk
                        rsp = psA.tile([128, 512], F32, tag="psA")
                        nc.tensor.matmul(rsp[:, :], ejS[:, j * 128:(j + 1) * 128],
                                         sAG[:, :], start=True, stop=True)
                        rsbS = apool.tile([128, 512], dt, tag="rsb", bufs=2)
                        nc.scalar.copy(rsbS[:, :], rsp[:, :])
                        xt = [apool.tile([128, 512], BF16, name=f"xt{k}", tag="xt",
                                         bufs=16) for k in range(8)]
                        for k in range(8):
                            nc.sync.dma_start(xt[k][:, :],
                                              xT[k * 128:(k + 1) * 128,
                                                 j * 512:(j + 1) * 512])
                        for m in range(3):
                            om = psB.tile([128, 512], F32, tag="psB")
                            for k in range(8):
                                nc.tensor.matmul(
                                    om[:, :],
                                    qkvW[:, k * 384 + m * 128:k * 384 + (m + 1) * 128],
                                    xt[k][:, :], start=(k == 0), stop=(k == 7))
                            if m == 0:
                                nc.vector.tensor_mul(
                                    Qf[:, blk * 512:(blk + 1) * 512],
                                    om[:, :], rsbS[:, :])
                            elif m == 1:
                                nc.vector.tensor_mul(
                                    Kf[:, blk * 512:(blk + 1) * 512],
                                    om[:, :], rsbS[:, :])
                            else:
                                vfb = apool.tile([128, 512], BF16, tag="vfb", bufs=2)
                                nc.vector.tensor_mul(vfb[:, :], om[:, :], rsbS[:, :])
                                for sub in range(4):
                                    kt = blk * 4 + sub
                                    for hh in range(2):
                                        vtp = psC.tile([128, 64], BF16, tag="psC")
                                        nc.tensor.transpose(
                                            vtp[:, :],
                                            vfb[64 * hh:64 * hh + 64,
                                                sub * 128:(sub + 1) * 128],
                                            identS[64 * hh:64 * hh + 64,
                                                   64 * hh:64 * hh + 64])
                                        nc.vector.tensor_copy(
                                            Vp[hh][kt][:, 0:64], vtp[:, :])

                    # attention for this batch's two heads
                    for hh in range(2):
                        r0 = 64 * hh
                        for qb in range(4):
                            q0 = qb * 512
                            ops = psC.tile([65, 512], F32, tag="psC")
                            nkt = 4 * qb + 4

                            def score_mm(kt):
                                sp = psB.tile([128, 512], F32, tag="psB")
                                nc.tensor.matmul(
                                    sp[:, :],
                                    Kf[r0:r0 + 64, kt * 128:(kt + 1) * 128],
                                    Qf[r0:r0 + 64, q0:q0 + 512],
                                    start=True, stop=True)
                                return sp

                            sps = score_mm(0)
                            for kt in range(nkt):
                                sp = sps
                                if kt + 1 < nkt:
                                    sps = score_mm(kt + 1)
                                e = apool.tile([128, 512], BF16, tag="expst", bufs=3)
                                d = kt - 4 * qb
                                if d < 0:
                                    nc.scalar.activation(e[:, :], sp[:, :], Act.Exp,
                                                         scale=0.125)
                                else:
                                    if d > 0:
                                        nc.vector.tensor_copy(e[:, 0:128 * d],
                                                              zerosF[:, 0:128 * d])
                                    nc.scalar.activation(e[:, 128 * d:512],
                                                         sp[:, 128 * d:512],
                                                         Act.Exp, scale=0.125)
                                    nc.vector.tensor_mul(
                                        e[:, 128 * d:128 * (d + 1)],
                                        e[:, 128 * d:128 * (d + 1)], triS[:, :])
                                nc.tensor.matmul(ops[:, :], Vp[hh][kt][:, :], e[:, :],
                                                 start=(kt == 0), stop=(kt == nkt - 1))
                            # 1/denominator = exp(-ln(den)) on Act engine
                            lnd = apool.tile([1, 512], F32, tag="lnd", bufs=2)
                            nc.scalar.activation(lnd[:, :], ops[64:65, :], Act.Ln)
                            rle = apool.tile([1, 512], dt, tag="rle", bufs=2)
                            nc.scalar.activation(rle[:, :], lnd[:, :], Act.Exp,
                                                 scale=-1.0)
                            rb = psA.tile([64, 512], F32, tag="psA")
                            nc.tensor.matmul(rb[:, :], onesS[0:1, 0:64], rle[:, :],
                                             start=True, stop=True)
                            rbc = apool.tile([64, 512], dt, tag="rbc", bufs=2)
                            nc.vector.tensor_copy(rbc[:, :], rb[:, :])
                            rbs = apool.tile([64, 512], BF16, tag="rbs", bufs=2)
                            nc.vector.tensor_mul(rbs[:, :], ops[0:64, :], rbc[:, :])
                            nc.sync.dma_start(cin[4 * b + qb, r0:r0 + 64, :],
                                              rbs[:, :])

            # post-phase weights: issue now, overlapping the A2A
            for k in range(8):
                nc.sync.dma_start(inW[:, k * SC:(k + 1) * SC],
                                  in_wT[k * 128:(k + 1) * 128, :])
                nc.sync.dma_start(gateW[:, k * SC:(k + 1) * SC],
                                  gate_wT[k * 128:(k + 1) * 128, :])
            nc.sync.dma_start(outW[:, :], out_wT[:, :])
            nc.sync.dma_start(dtW[:, :], dt_wT[:, :])
            nc.sync.dma_start(BpS[:, :], BpT[:, :])
            nc.sync.dma_start(CpS[:, :], CpT[:, :])

            nc.gpsimd.collective_compute(
                "AllToAll", Alu.bypass, [list(range(NC))],
                ins=[cin.opt()], outs=[cout.opt()])

            with tc.tile_pool(name="xpool", bufs=1) as xpool:
                x1d = [xpool.tile([128, 512], dt, name=f"x1d{m}", tag=f"x1d{m}")
                       for m in range(8)]
                h2T = [xpool.tile([128, 512], dt, name=f"h2T{k}", tag=f"h2T{k}")
                       for k in range(8)]

                with tc.tile_pool(name="postA", bufs=1) as pA:
                    # attention rows for my tokens, loaded once (bf16)
                    otg = [pA.tile([128, 512], BF16, name=f"otg{k}", tag=f"otg{k}")
                           for k in range(8)]
                    for k in range(8):
                        nc.sync.dma_start(otg[k][:, :], cout[k, :, :])
                    # o_proj + residual (D-major)
                    for m in range(8):
                        pr = psB.tile([128, 512], F32, tag="psB")
                        for k in range(8):
                            nc.tensor.matmul(
                                pr[:, :],
                                oW[:, k * D + m * 128:k * D + (m + 1) * 128],
                                otg[k][:, :], start=(k == 0), stop=(k == 7))
                        nc.vector.tensor_add(x1d[m][:, :], pr[:, :], xtmy[m][:, :])

                # ---- rmsnorm2 (D-major) ----
                ssp2 = psA.tile([1, 512], F32, tag="psA")
                for k in range(8):
                    sq2 = work.tile([128, 512], dt, tag="sq", bufs=2)
                    nc.scalar.activation(sq2[:, :], x1d[k][:, :], Act.Square)
                    nc.tensor.matmul(ssp2[:, :], onesS[:, 0:1], sq2[:, :],
                                     start=(k == 0), stop=(k == 7))
                ln2 = work.tile([1, 512], F32, tag="lnm", bufs=1)
                nc.scalar.activation(ln2[:, :], ssp2[:, :], Act.Ln,
                                     scale=1.0 / D, bias=epsS[0:1, :])
                s2row = work.tile([1, 512], dt, tag="srow", bufs=1)
                nc.scalar.activation(s2row[:, :], ln2[:, :], Act.Exp, scale=-0.5)
                rs2p = psA.tile([128, 512], F32, tag="psA")
                nc.tensor.matmul(rs2p[:, :], onesS[0:1, 0:128], s2row[:, :],
                                 start=True, stop=True)
                rsb2S = work.tile([128, 512], dt, tag="rsb2", bufs=1)
                nc.scalar.copy(rsb2S[:, :], rs2p[:, :])
                for k in range(8):
                    nc.vector.tensor_mul(h2T[k][:, :], x1d[k][:, :], rsb2S[:, :])

                with tc.tile_pool(name="postB", bufs=1) as pB:
                    # ---- scan projections ----
                    pz = psB.tile([SC, 512], F32, tag="psB")
                    for k in range(8):
                        nc.tensor.matmul(pz[:, :], inW[:, k * SC:(k + 1) * SC],
                                         h2T[k][:, :], start=(k == 0), stop=(k == 7))
                    z_s = pB.tile([SC, 512], dt, name="z_s", tag="z_s")
                    nc.vector.tensor_copy(z_s[:, :], pz[:, :])
                    pdt = psB.tile([SC, 512], F32, tag="psB")
                    nc.tensor.matmul(pdt[:, :], dtW[:, :], z_s[:, :],
                                     start=True, stop=True)
                    dt_s = pB.tile([SC, 512], dt, name="dt_s", tag="dt_s")
                    nc.scalar.activation(dt_s[:, :], pdt[:, :], Act.Exp,
                                         bias=dtbS[:, :])
                    nc.scalar.activation(dt_s[:, :], dt_s[:, :], Act.Ln, bias=1.0)
                    dtz_s = pB.tile([SC, 512], dt, name="dtz_s", tag="dtz_s")
                    nc.vector.tensor_mul(dtz_s[:, :], dt_s[:, :], z_s[:, :])
                    pbi = psB.tile([ST, 512], F32, tag="psB")
                    nc.tensor.matmul(pbi[:, :], BpS[:, :], z_s[:, :],
                                     start=True, stop=True)
                    bi_s = pB.tile([ST, 512], dt, name="bi_s", tag="bi_s")
                    nc.vector.tensor_copy(bi_s[:, :], pbi[:, :])
                    pci = psB.tile([ST, 512], F32, tag="psB")
                    nc.tensor.matmul(pci[:, :], CpS[:, :], z_s[:, :],
                                     start=True, stop=True)
                    ci_s = pB.tile([ST, 512], dt, name="ci_s", tag="ci_s")
                    nc.vector.tensor_copy(ci_s[:, :], pci[:, :])

                    # ---- local scans (zero-init trajectories + cumprods) ----
                    ppT = [pB.tile([128, 512], BF16, name=f"ppT{g}", tag=f"ppT{g}")
                           for g in range(8)]
                    sc0T = [pB.tile([128, 512], dt, name=f"sc0T{g}", tag=f"sc0T{g}")
                            for g in range(8)]
                    stg2 = pB.tile([128, 16], F32, name="stg2", tag="stg2")
                    for g in range(8):
                        pde = psB.tile([128, 512], F32, tag="psB")
                        nc.tensor.matmul(pde[:, :], escS[:, g * 128:(g + 1) * 128],
                                         dt_s[:, :], start=True, stop=True)
                        abar = pB.tile([128, 512], dt, tag="abar", bufs=2)
                        nc.scalar.activation(abar[:, :], pde[:, :], Act.Identity,
                                             scale=negA[:, g:g + 1], bias=1.0)
                        pdz = psB.tile([128, 512], F32, tag="psB")
                        nc.tensor.matmul(pdz[:, :], escS[:, g * 128:(g + 1) * 128],
                                         dtz_s[:, :], start=True, stop=True)
                        pbe = psC.tile([128, 512], F32, tag="psC")
                        nc.tensor.matmul(pbe[:, :], estS[:, :], bi_s[:, :],
                                         start=True, stop=True)
                        bes = pB.tile([128, 512], dt, tag="bes", bufs=2)
                        nc.scalar.copy(bes[:, :], pbe[:, :])
                        bin_ = pB.tile([128, 512], dt, tag="bin_", bufs=2)
                        nc.vector.tensor_mul(bin_[:, :], pdz[:, :], bes[:, :])
                        nc.vector.tensor_tensor_scan(sc0T[g][:, :], abar[:, :],
                                                     bin_[:, :], 0.0,
                                                     Alu.mult, Alu.add)
                        nc.vector.tensor_tensor_scan(ppT[g][:, :], abar[:, :],
                                                     abar[:, :], 1.0,
                                                     Alu.mult, Alu.bypass)
                        nc.vector.tensor_copy(stg2[:, g:g + 1],
                                              ppT[g][:, 511:512])
                        nc.vector.tensor_copy(stg2[:, 8 + g:8 + g + 1],
                                              sc0T[g][:, 511:512])

                    nc.gpsimd.dma_start(cin2[:, :], stg2[:, :])
                    nc.gpsimd.collective_compute(
                        "AllGather", Alu.bypass, [list(range(NC))],
                        ins=[cin2.opt()], outs=[cout2.opt()])

                    # overlap the AllGather with work that doesn't need it:
                    # C embedding and the gate (single act-table switch to Silu)
                    ces = pB.tile([128, 512], dt, name="ces", tag="ces")
                    pce = psC.tile([128, 512], F32, tag="psC")
                    nc.tensor.matmul(pce[:, :], estS[:, :], ci_s[:, :],
                                     start=True, stop=True)
                    nc.vector.tensor_copy(ces[:, :], pce[:, :])
                    pg = psB.tile([SC, 512], F32, tag="psB")
                    for k in range(8):
                        nc.tensor.matmul(pg[:, :], gateW[:, k * SC:(k + 1) * SC],
                                         h2T[k][:, :], start=(k == 0), stop=(k == 7))
                    gate_s = pB.tile([SC, 512], dt, name="gate_s", tag="gate_s")
                    nc.scalar.activation(gate_s[:, :], pg[:, :], Act.Silu)

                    # ---- stitch initial states from preceding cores ----
                    sin = pB.tile([128, 8], F32, name="sin", tag="sin")
                    nc.vector.memset(sin[:, :], 0.0)
                    for jj in range(NC):
                        pj = work.tile([128, 16], F32, tag="pj")
                        nc.sync.dma_start(pj[:, :],
                                          cout2[jj * 128:(jj + 1) * 128, :])
                        pe_ = work.tile([128, 8], F32, tag="pe_")
                        nc.vector.scalar_tensor_tensor(pe_[:, :], pj[:, 0:8],
                                                       cselS[:, jj:jj + 1],
                                                       omcS[:, 8 * jj:8 * jj + 8],
                                                       Alu.mult, Alu.add)
                        se_ = work.tile([128, 8], F32, tag="se_")
                        nc.vector.tensor_scalar_mul(se_[:, :], pj[:, 8:16],
                                                    cselS[:, jj:jj + 1])
                        nc.vector.tensor_mul(sin[:, :], sin[:, :], pe_[:, :])
                        nc.vector.tensor_add(sin[:, :], sin[:, :], se_[:, :])

                    # ---- finalize: state = sc0 + sin * pp  (linearity) ----
                    py = psA.tile([SC, 512], F32, tag="psA")
                    for g in range(8):
                        stc = pB.tile([128, 512], dt, tag="stc", bufs=2)
                        nc.vector.scalar_tensor_tensor(stc[:, :], ppT[g][:, :],
                                                       sin[:, g:g + 1],
                                                       sc0T[g][:, :],
                                                       Alu.mult, Alu.add)
                        yt = pB.tile([128, 512], dt, tag="yt", bufs=2)
                        nc.vector.tensor_mul(yt[:, :], stc[:, :], ces[:, :])
                        nc.tensor.matmul(py[:, :], r8S[:, g * 64:(g + 1) * 64],
                                         yt[:, :], start=(g == 0), stop=(g == 7))
                    yT = pB.tile([SC, 512], dt, name="yT", tag="yT")
                    nc.vector.tensor_copy(yT[:, :], py[:, :])

                    # ---- gate + out_proj + final residual (D-major) ----
                    yg = pB.tile([SC, 512], dt, name="yg", tag="yg")
                    nc.vector.tensor_mul(yg[:, :], yT[:, :], gate_s[:, :])
                    for m in range(8):
                        p2 = psB.tile([128, 512], F32, tag="psB")
                        nc.tensor.matmul(p2[:, :], outW[:, m * 128:(m + 1) * 128],
                                         yg[:, :], start=True, stop=True)
                        yo = pB.tile([128, 512], dt, tag="yo", bufs=2)
                        nc.vector.tensor_add(yo[:, :], p2[:, :], x1d[m][:, :])
                        nc.sync.dma_start(yout[m * 128:(m + 1) * 128, :], yo[:, :])

    _split_multi_waits(nc)
    return nc


def kernel(x, qkv_w, o_w, norm1_w, norm2_w, in_w, out_w, A_log, Bp_w, Cp_w,
           dt_w, dt_b, gate_w):
    import ml_dtypes
    f = np.float32
    bf = ml_dtypes.bfloat16
    xf = np.ascontiguousarray(np.asarray(x, f).reshape(BT, D))
    xT_f = np.ascontiguousarray(xf.T)
    xT = xT_f.astype(bf)
    # fold the rmsnorm elementwise weights into the consuming projections
    qkv_w1 = np.asarray(qkv_w, f) * np.asarray(norm1_w, f)[None, :]
    in_w2 = np.asarray(in_w, f) * np.asarray(norm2_w, f)[None, :]
    gate_w2 = np.asarray(gate_w, f) * np.asarray(norm2_w, f)[None, :]
    o_wTb = np.ascontiguousarray(np.asarray(o_w, f).T.astype(bf))
    out_wT = np.ascontiguousarray(np.asarray(out_w, f).T)
    in_wT = np.ascontiguousarray(in_w2.T)
    gate_wT = np.ascontiguousarray(gate_w2.T)
    dt_wT = np.ascontiguousarray(np.asarray(dt_w, f).T)
    BpT = np.ascontiguousarray(np.asarray(Bp_w, f).T)
    CpT = np.ascontiguousarray(np.asarray(Cp_w, f).T)
    dtbv = np.ascontiguousarray(np.asarray(dt_b, f).reshape(SC, 1))
    alogv = np.ascontiguousarray(np.asarray(A_log, f).reshape(1024).reshape(8, 128).T)
    ident = np.eye(128, dtype=f).astype(bf)
    tri_m = (np.arange(128)[None, :] >= np.arange(128)[:, None]).astype(f).astype(bf)
    onesd = np.ones((128, 128), f)
    ejm = np.zeros((8, 1024), f)
    for j in range(8):
        ejm[j, j * 128:(j + 1) * 128] = 1.0
    jj = np.arange(1024)
    escm = (np.arange(SC)[:, None] == (jj[None, :] // 16)).astype(f)
    estm = (np.arange(ST)[:, None] == (np.arange(128)[None, :] % 16)).astype(f)
    r8m = np.zeros((128, 512), f)
    for g in range(8):
        for j in range(128):
            r8m[j, g * 64 + 8 * g + j // 16] = 1.0

    nc = _build()
    in_maps = []
    for c in range(NC):
        b, q = c // 4, c % 4
        h0 = 2 * c
        rows = np.concatenate([np.arange(h0 * 64, (h0 + 2) * 64),
                               D + np.arange(h0 * 64, (h0 + 2) * 64),
                               2 * D + np.arange(h0 * 64, (h0 + 2) * 64)])
        qkvwT = np.ascontiguousarray(qkv_w1[rows, :].T.astype(bf))
        xmyT = np.ascontiguousarray(xT_f[:, c * TOK:(c + 1) * TOK])
        sel = np.zeros(NC, f)
        for j in range(q):
            sel[4 * b + j] = 1.0
        cselv = np.ascontiguousarray(np.tile(sel[None, :], (128, 1)))
        omcv = np.ascontiguousarray(
            np.repeat(1.0 - sel, 8)[None, :].repeat(128, axis=0).astype(f))
        in_maps.append({
            "xT": xT, "xmyT": xmyT, "qkvwT": qkvwT, "o_wTb": o_wTb,
            "out_wT": out_wT, "in_wT": in_wT, "gate_wT": gate_wT,
            "dt_wT": dt_wT, "BpT": BpT, "CpT": CpT,
            "dtb": dtbv, "alog": alogv, "ident": ident, "tri": tri_m,
            "onesd": onesd, "ej": ejm, "esc": escm, "est": estm, "r8": r8m,
            "csel": cselv, "omc": omcv,
            "epsb": np.full((128, 1), EPS, f),
        })
    import os
    trace = bool(int(os.environ.get("BASS_PROFILE", "0")))
    res = run_bass_kernel_spmd(nc, in_maps, core_ids=list(range(NC)),
                               trace=trace)
    if trace:
        print("HW exec time:", res.exec_time_ns, "ns")
        print("trace:", res.instructions_and_trace[1] if res.instructions_and_trace else None)
    out = np.concatenate([res.results[c]["yout"].T for c in range(NC)], axis=0)
    return out.reshape(B, T, D)
